# revision 54
# baseline (speedup 1.0000x reference)
"""Causal multi-head attention (RoPE) on 8 TRN2 NeuronCores.

Problem: x[2,2048,2048] -> qkv proj -> rope -> causal attention (16 heads,
head_dim 128) -> output proj + bias. Sharding: (batch, head-group) across the
8 cores - core c handles batch c//4 and heads 4*(c%4)..4*(c%4)+3. Each core
computes a partial output projection over its heads' channels; the host sums
the 4 partials per batch and adds b_o.

Single-pass token-outer pipeline, everything fp16 on device (PSUM accumulation
stays f32; final host reduction in f32; validated rel err ~5e-4 vs the fp32
reference). The exp throughput on ACT (0.833ns/col) exactly matches the
scores+AV cost on PE, so attention phases are ACT-bound unless PE borrows
other work: the next block's QKV projection is software-pipelined INTO the
attention window as three waves:

    attn(nb) heads -> q-wave(nb+1) -> outproj(nb) -> k-wave(nb+1)
                   -> v-wave(nb+1) -> attn(nb+1) ...

Waves are accumulator-major (16 kt matmuls per PSUM bank) with a bank map
chosen so each wave's first banks were freed earliest by the previous phase:
q-wave on b1,b2,b0,b7 / k-wave on b3..b6 / v-wave on b0,b7,b1,b2; attention
rotates scores over b3..b6 4-deep (tag b{3+(i+h)%4}), softmax-denominator
broadcast lb takes the next slot in that rotation, ctx alternates b0/b7.

Scores are transposed s^T[tk,tq] (lhsT=k tile, rhs=q block) with causal
narrowing; matmul cost here is (moving columns) x (cycles/row keyed on the
MOVING operand dtype): fp16 runs 1 cycle/row with no 256-column minimum, so
the r=3 diagonal tile narrows to 128 columns. Softmax denominators come from
element-wise fp16 accumulation of the exp tiles on DVE (2x mode) + ONE
ones-matmul per (head, block) that broadcasts the partition sum - the
per-tile [1,512] ones-matmuls this replaces cost a full 30us of PE. RoPE is
applied in place (half-swap via 2 small SBUF DMAs, sign folded into sinT on
the host; mults split Pool/DVE). Output projection accumulates the 4 heads
in PSUM per 128-token sub-tile; quarter evictions alternate DVE/ACT into an
fp16 [128,2048] staging row, one DMA per sub-tile (per-quarter DMAs on the
last block to shorten the tail). DMAs are batched multi-kt loads; x for block
nb+1 prefetches during attention nb.
"""
import math

import numpy as np

import concourse.bacc as bacc
import concourse.mybir as mybir
import concourse.tile as tile
from concourse.bass_utils import run_bass_kernel_spmd

P = 128           # partitions / head_dim
T = 2048          # context length
C = 2048          # d_model
NKT = C // P      # 16 contraction tiles
NB = T // 512     # 4 token blocks of 512
HPC = 4           # heads per core
NCORES = 8
SCALE = 1.0 / math.sqrt(P)

F32 = mybir.dt.float32
F16 = mybir.dt.float16
EXP = mybir.ActivationFunctionType.Exp
MULT = mybir.AluOpType.mult
ADD = mybir.AluOpType.add

QBANKS = ("b1", "b2", "b0", "b7")   # q-wave accumulators, emission order
KBANKS = ("b3", "b4", "b5", "b6")   # k-wave accumulators
VBANKS = ("b0", "b7", "b1", "b2")   # v-wave accumulators
CTXBANKS = ("b0", "b7")             # ctx_ps alternates by head parity

_CACHE = {}


def _build():
    nc = bacc.Bacc("TRN2", target_bir_lowering=False, debug=False,
                   num_devices=NCORES)
    xg = nc.dram_tensor("xg", (P, NKT, T), F16, kind="ExternalInput").ap()
    wq = nc.dram_tensor("wq", (P, NKT, HPC * P), F16, kind="ExternalInput").ap()
    wk = nc.dram_tensor("wk", (P, NKT, HPC * P), F16, kind="ExternalInput").ap()
    wv = nc.dram_tensor("wv", (P, NKT, HPC * P), F16, kind="ExternalInput").ap()
    wo = nc.dram_tensor("wo", (P, HPC, C), F16, kind="ExternalInput").ap()
    cosT = nc.dram_tensor("cosT", (P, T), F16, kind="ExternalInput").ap()
    sinT = nc.dram_tensor("sinT", (P, T), F16, kind="ExternalInput").ap()
    tri = nc.dram_tensor("tri", (P, P), F16, kind="ExternalInput").ap()
    ones = nc.dram_tensor("ones", (P, P), F16, kind="ExternalInput").ap()
    eye = nc.dram_tensor("eye", (P, P), F16, kind="ExternalInput").ap()
    y = nc.dram_tensor("y", (T, C), F16, kind="ExternalOutput").ap()

    half = P // 2

    with tile.TileContext(nc) as tc:
        with (
            tc.tile_pool(name="gconst", bufs=1) as gpool,
            tc.tile_pool(name="wbuf", bufs=1) as wpool,
            tc.tile_pool(name="xbuf", bufs=1) as xpool,
            tc.tile_pool(name="qkbuf", bufs=1) as qkpool,
            tc.tile_pool(name="vbuf", bufs=1) as vpool,
            tc.tile_pool(name="rope", bufs=1) as rpool,
            tc.tile_pool(name="ptb", bufs=1) as ptpool,
            tc.tile_pool(name="stats", bufs=1) as spool,
            tc.tile_pool(name="ctxb", bufs=1) as cxpool,
            tc.tile_pool(name="yb", bufs=1) as ypool,
            tc.tile_pool(name="ps", bufs=1, space="PSUM") as ps,
        ):
            tri_sb = gpool.tile([P, P], F16, tag="tri")
            ones_sb = gpool.tile([P, P], F16, tag="ones")
            eye_sb = gpool.tile([P, P], F16, tag="eye")
            wq_sb = wpool.tile([P, NKT, HPC * P], F16, tag="wq", name="wq_sb")
            wk_sb = wpool.tile([P, NKT, HPC * P], F16, tag="wk", name="wk_sb")
            wv_sb = wpool.tile([P, NKT, HPC * P], F16, tag="wv", name="wv_sb")
            wo_sb = wpool.tile([P, HPC, C], F16, tag="wo", name="wo_sb")
            cos_sb = wpool.tile([P, T], F16, tag="cos", name="cos_sb")
            sin_sb = wpool.tile([P, T], F16, tag="sin", name="sin_sb")

            qk_sb = {}
            for h in range(HPC):
                for part in ("q", "k"):
                    for nb in range(NB):
                        qk_sb[(part, h, nb)] = qkpool.tile(
                            [P, 512], F16, tag=f"{part}{h}n{nb}",
                            name=f"{part}{h}n{nb}_sb")
            v_sb = [vpool.tile([P, 512], F16, tag=f"vb{i}", name=f"v{i}_sb")
                    for i in range(NKT)]



            def wave_accs(banks, label):
                return [ps.tile([P, 512], F32, tag=banks[h],
                                name=f"{label}{h}") for h in range(HPC)]

            def rope_one(nb, part, h):
                """In-place rope on an evicted q/k chunk. Emitted away from
                the attention masks: the Pool t1 multiply is 1.1us, and a
                diagonal mask queued behind it stalls the AV matmuls."""
                nsl = slice(nb * 512, (nb + 1) * 512)
                dst = qk_sb[(part, h, nb)]
                tmp = rpool.tile([P, 512], F16, tag="rt", bufs=2, name="rtmp")
                nc.sync.dma_start(tmp[0:half, :], dst[half:P, :])
                nc.sync.dma_start(tmp[half:P, :], dst[0:half, :])
                t1 = rpool.tile([P, 512], F16, tag="t1", bufs=2)
                nc.gpsimd.tensor_tensor(t1[:], dst[:], cos_sb[:, nsl], op=MULT)
                t2 = rpool.tile([P, 512], F16, tag="t2", bufs=2)
                nc.vector.tensor_tensor(t2[:], tmp[:], sin_sb[:, nsl], op=MULT)
                nc.vector.tensor_tensor(dst[:], t1[:], t2[:], op=ADD)

            def evict_rope_one(nb, part, h, acc_ap):
                nc.scalar.copy(qk_sb[(part, h, nb)][:], acc_ap)
                rope_one(nb, part, h)

            def qk_wave_chain(part, xt, h, bank):
                wsb = wq_sb if part == "q" else wk_sb
                acc = ps.tile([P, 512], F32, tag=bank, name=f"{part}{h}")
                for kt in range(NKT):
                    nc.tensor.matmul(
                        acc[:], wsb[:, kt, h * P:(h + 1) * P],
                        xt[:, kt, :], start=(kt == 0), stop=(kt == NKT - 1))
                return acc

            def v_chain(nb, tt, xt, bank):
                vacc = ps.tile([P, 512], F32, tag=bank, name=f"v{nb}_{tt}")
                for kt in range(NKT):
                    nc.tensor.matmul(
                        vacc[:], xt[:, kt, tt * P:(tt + 1) * P],
                        wv_sb[:, kt, :], start=(kt == 0), stop=(kt == NKT - 1))
                return vacc

            def attention(nb, xt_next, vaccs, partA=None):
                """vaccs: this block's un-evicted v accumulators (tt 1..3 on
                b1,b2,b7); evictions are emitted just-in-time at the diagonal
                steps of head 0 so ACT serves head 0's first exps first. For
                nb==3 the tt>=1 v chains are emitted inside head 0 as PE
                filler (no next-block waves exist). Head h's softmax stats
                (lb matmul, reciprocal, normalize) are deferred into head
                h+1's pipeline so PE never waits on the DVE denominator
                chain. For nb<3 the next block's q-wave chain for head h is
                emitted right after head h (banks b1/b2 alternating, evicted
                and roped immediately)."""
                nt = 4 * (nb + 1)
                ctx_tiles = {}
                pending = None

                def q_filler():
                    # next block's q projection, one matmul per drain unit,
                    # each chain evicted (ACT) as soon as it completes
                    for fh in range(HPC):
                        acc = ps.tile([P, 512], F32, tag=f"b{1 + fh % 2}",
                                      name=f"q{fh}")
                        for kt in range(NKT):
                            nc.tensor.matmul(
                                acc[:], wq_sb[:, kt, fh * P:(fh + 1) * P],
                                xt_next[:, kt, :], start=(kt == 0),
                                stop=(kt == NKT - 1))
                            yield
                        nc.scalar.copy(qk_sb[("q", fh, nb + 1)][:], acc[:])

                filler = q_filler() if nb < NB - 1 else None

                def drain(n):
                    if filler is None:
                        return
                    for _ in range(n):
                        if next(filler, "done") == "done":
                            break

                per_step = max(1, 58 // (3 * nt))

                def stats(h, ctx_ps, lacc):
                    # slot (h+3)%4 is the one head h+1 touches last after the
                    # deferred emission point, so the reciprocal drains
                    # before the bank is needed again
                    lbt = ps.tile([P, 512], F32, tag=f"b{3 + (h + 3) % 4}",
                                  name=f"l{h}_{nb}")
                    nc.tensor.matmul(lbt[:], ones_sb[:], lacc[:],
                                     start=True, stop=True)
                    rinv = spool.tile([P, 512], F32, tag="rinv", bufs=2)
                    ctx_sb = cxpool.tile([P, 512], F16, tag=f"cx{h}", bufs=2,
                                         name=f"cs{h}_{nb}")
                    # last head's normalize gates the output projection: do it
                    # in chunks so outproj's first sub-tile unblocks early
                    for lo, hi in ((0, 128), (128, 256), (256, 512)) \
                            if h == HPC - 1 else ((0, 512),):
                        nc.vector.reciprocal(rinv[:, lo:hi], lbt[:, lo:hi])
                        nc.vector.tensor_tensor(ctx_sb[:, lo:hi],
                                                ctx_ps[:, lo:hi],
                                                rinv[:, lo:hi], op=MULT)
                    ctx_tiles[h] = ctx_sb

                i0 = 8 if partA is not None else 0
                pend_at = i0 + (4 if nt - i0 > 4 else 2)
                for h in range(HPC):
                    qT = qk_sb[("q", h, nb)]
                    ctx_ps = ps.tile([P, 512], F32, tag=CTXBANKS[h % 2],
                                     name=f"ctx{h}_{nb}")
                    lacc = spool.tile([P, 512], F16, tag="lacc", bufs=2)
                    if partA is not None:
                        # resume this head's accumulation from the partial
                        # computed in the wave window: identity-matmul the
                        # parked ctx back into PSUM (opens the group), start
                        # the denominator from the parked lacc
                        ctxA, laccA = partA
                        nc.tensor.matmul(ctx_ps[:], eye_sb[:], ctxA[h][:],
                                         start=True, stop=False)
                        nc.vector.tensor_copy(lacc[:], laccA[h][:])
                    for i in range(i0, nt):
                        r = i - 4 * nb
                        if h == 0 and r >= 1 and r in vaccs:
                            nc.scalar.copy(v_sb[nb * 4 + r][:], vaccs[r][:])
                        if h > 0 and i == pend_at and pending is not None:
                            stats(*pending)
                            pending = None
                        c0 = 0 if r < 1 else r * P
                        osl = slice(c0, 512)
                        sps = ps.tile([P, 512], F32, tag=f"b{3 + (i + h) % 4}",
                                      name=f"s{h}_{nb}_{i}")
                        nc.tensor.matmul(
                            sps[:, osl],
                            qk_sb[("k", h, i // 4)][:,
                                                    (i % 4) * P:(i % 4 + 1) * P],
                            qT[:, osl], start=True, stop=True)
                        pt = ptpool.tile([P, 512], F16, tag="pt", bufs=6)
                        nc.scalar.activation(pt[:, osl], sps[:, osl], EXP,
                                             scale=SCALE)
                        if r >= 0:
                            # diagonal mask on DVE: fp16 2x mode takes 127ns
                            # vs Pool's 444ns+launch, and it's on the exp->AV
                            # critical path every diagonal step
                            dsl = slice(r * P, (r + 1) * P)
                            nc.vector.tensor_tensor(
                                pt[:, dsl], pt[:, dsl], tri_sb[:], op=MULT)
                        nc.tensor.matmul(
                            ctx_ps[:, osl],
                            v_sb[i][:, h * P:(h + 1) * P], pt[:, osl],
                            start=(i == 0 and partA is None),
                            stop=(i == nt - 1))
                        if i == i0 and partA is None:
                            nc.vector.tensor_copy(lacc[:], pt[:])
                        else:
                            nc.vector.tensor_tensor(
                                lacc[:, osl], lacc[:, osl], pt[:, osl], op=ADD)
                        if h >= 1:
                            drain(per_step)
                    if h == HPC - 1:
                        if pending is not None:
                            stats(*pending)
                            pending = None
                        drain(6)
                        stats(h, ctx_ps, lacc)
                        drain(NKT * HPC)
                    else:
                        pending = (h, ctx_ps, lacc)
                if nb < NB - 1:
                    for h in range(HPC):
                        rope_one(nb + 1, "q", h)
                return ctx_tiles

            def outproj(nb, ctx_tiles):
                last = nb == NB - 1

                def make_yps(ob, name):
                    # 4-bank ring absorbs eviction latency; b1 frees earliest
                    # for the k-wave that follows
                    t = ps.tile([P, 512], F32, tag=("b1", "b2", "b3", "b4")[ob],
                                name=name)
                    return lambda lo=0, hi=512: t[:, lo:hi]

                for sub in range(4):
                    tt = nb * 4 + sub
                    ssl = slice(sub * P, (sub + 1) * P)
                    ysb = ypool.tile([P, C], F16, tag="ysb", bufs=3)
                    ypsf = {}
                    if last and sub == 0:
                        # on the final block the h3 normalize is still in
                        # flight; open all 4 accumulation groups with their
                        # h0..h2 contributions first to cover its latency
                        for ob in range(NB):
                            obsl = slice(ob * 512, (ob + 1) * 512)
                            ypsf[ob] = make_yps(ob, f"y{tt}_{ob}")
                            for h in range(HPC - 1):
                                nc.tensor.matmul(
                                    ypsf[ob](), ctx_tiles[h][:, ssl],
                                    wo_sb[:, h, obsl],
                                    start=(h == 0), stop=False,
                                    skip_group_check=True)
                    for ob in range(NB):
                        obsl = slice(ob * 512, (ob + 1) * 512)
                        if ob in ypsf:
                            f = ypsf[ob]
                            nc.tensor.matmul(
                                f(), ctx_tiles[HPC - 1][:, ssl],
                                wo_sb[:, HPC - 1, obsl],
                                start=False, stop=True,
                                skip_group_check=True)
                        else:
                            f = make_yps(ob, f"y{tt}_{ob}")
                            halves = ((0, 512),)
                            if last and sub == 3 and ob == 3:
                                # split the final group so its first half's
                                # eviction/DMA overlaps the second half
                                halves = ((0, 256), (256, 512))
                            for lo, hi in halves:
                                for h in range(HPC):
                                    nc.tensor.matmul(
                                        f(lo, hi), ctx_tiles[h][:, ssl],
                                        wo_sb[:, h, ob * 512 + lo:
                                              ob * 512 + hi],
                                        start=(h == 0), stop=(h == HPC - 1),
                                        skip_group_check=True)
                                if hi - lo < 512:
                                    nc.scalar.copy(
                                        ysb[:, ob * 512 + lo:ob * 512 + hi],
                                        f(lo, hi))
                                    nc.sync.dma_start(
                                        y[tt * P:(tt + 1) * P,
                                          ob * 512 + lo:ob * 512 + hi],
                                        ysb[:, ob * 512 + lo:ob * 512 + hi])
                        if last and sub == 3 and ob == 3:
                            continue
                        if ob % 2 == 0 and not (last and sub == 3):
                            nc.vector.tensor_copy(ysb[:, obsl], f())
                        else:
                            # ACT is idle by the end of the last block
                            nc.scalar.copy(ysb[:, obsl], f())
                        if last:
                            nc.sync.dma_start(
                                y[tt * P:(tt + 1) * P, obsl], ysb[:, obsl])
                    if not last:
                        nc.sync.dma_start(y[tt * P:(tt + 1) * P, :], ysb[:])

            # ---- startup DMAs: x(0)+wq+wk interleaved, fine-grained head ----
            x0 = xpool.tile([P, NKT, 512], F16, tag="x", bufs=2, name="x_nb0")
            groups = [(0, 1), (1, 2), (2, 4), (4, 8), (8, 12), (12, 16)]
            for lo, hi in groups:
                gs = slice(lo, hi)
                nc.sync.dma_start(x0[:, gs, :], xg[:, gs, 0:512])
                nc.sync.dma_start(wq_sb[:, gs, :], wq[:, gs, :])
                nc.sync.dma_start(wk_sb[:, gs, :], wk[:, gs, :])
            nc.sync.dma_start(cos_sb[:], cosT)
            nc.sync.dma_start(sin_sb[:], sinT)
            nc.sync.dma_start(wv_sb[:], wv)
            nc.sync.dma_start(tri_sb[:], tri)
            nc.sync.dma_start(ones_sb[:], ones)
            nc.sync.dma_start(eye_sb[:], eye)
            nc.sync.dma_start(wo_sb[:], wo)

            def v_group(nbv, xt_src):
                """tt0 evicted immediately; tt1..3 stay in PSUM for the JIT
                evictions inside attention(nbv) head 0."""
                va0 = v_chain(nbv, 0, xt_src, "b1")
                nc.scalar.copy(v_sb[nbv * 4][:], va0[:])
                return {tt: v_chain(nbv, tt, xt_src, ("b1", "b2", "b7")[tt - 1])
                        for tt in (1, 2, 3)}

            # ---- block 0: plain qk (kt-outer tracks DMA arrival), v ----
            qaccs = wave_accs(QBANKS, "q")
            kaccs = wave_accs(KBANKS, "k")
            for kt in range(NKT):
                for part, wsb, accs in (("q", wq_sb, qaccs),
                                        ("k", wk_sb, kaccs)):
                    for h in range(HPC):
                        nc.tensor.matmul(
                            accs[h][:], wsb[:, kt, h * P:(h + 1) * P],
                            x0[:, kt, :], start=(kt == 0), stop=(kt == NKT - 1))
            for h in range(HPC):
                evict_rope_one(0, "k", h, kaccs[h][:])
            for h in range(HPC):
                evict_rope_one(0, "q", h, qaccs[h][:])
            vaccs = v_group(0, x0)

            xt = x0
            partA_cur = None
            for nb in range(NB):
                # prefetch next block's x during attention
                if nb < NB - 1:
                    xt_next = xpool.tile([P, NKT, 512], F16, tag="x", bufs=2,
                                         name=f"x_nb{nb + 1}")
                    nsl2 = slice((nb + 1) * 512, (nb + 2) * 512)
                    # halves: the first q-filler chains only need low kt
                    nc.sync.dma_start(xt_next[:, 0:8, :], xg[:, 0:8, nsl2])
                    nc.sync.dma_start(xt_next[:, 8:16, :], xg[:, 8:16, nsl2])
                else:
                    xt_next = xt
                ctx_tiles = attention(nb, xt_next, vaccs, partA=partA_cur)
                outproj(nb, ctx_tiles)
                if nb < NB - 1:
                    if nb + 1 < NB - 1:
                        for h in range(HPC):
                            ka = qk_wave_chain("k", xt_next, h,
                                               f"b{1 + h % 2}")
                            evict_rope_one(nb + 1, "k", h, ka[:])
                        vaccs = v_group(nb + 1, xt_next)
                    else:
                        # the last block's attention is ACT(exp)-bound, so
                        # compute its first two key-blocks per head NOW,
                        # interleaved with the k-wave, where ACT has slack;
                        # partial ctx/lacc park in SBUF (fp16) and resume in
                        # attention(3) via an identity-matmul preload
                        ctxA, laccA = {}, {}
                        for h in range(HPC):
                            ka = qk_wave_chain("k", xt_next, h,
                                               f"b{1 + h % 2}")
                            evict_rope_one(nb + 1, "k", h, ka[:])
                            qT = qk_sb[("q", h, nb + 1)]
                            cA = ps.tile([P, 512], F32, tag=CTXBANKS[h % 2],
                                         name=f"cA{h}")
                            lA = spool.tile([P, 512], F16, tag=f"lA{h}",
                                            bufs=1)
                            for i in range(8):
                                sA = ps.tile([P, 512], F32,
                                             tag=f"b{3 + (i + h) % 4}",
                                             name=f"sA{h}_{i}")
                                nc.tensor.matmul(
                                    sA[:],
                                    qk_sb[("k", h, i // 4)][:,
                                                            (i % 4) * P:
                                                            (i % 4 + 1) * P],
                                    qT[:], start=True, stop=True)
                                pt = ptpool.tile([P, 512], F16, tag="pt",
                                                 bufs=6)
                                nc.scalar.activation(pt[:], sA[:], EXP,
                                                     scale=SCALE)
                                nc.tensor.matmul(
                                    cA[:], v_sb[i][:, h * P:(h + 1) * P],
                                    pt[:], start=(i == 0), stop=(i == 7))
                                if i == 0:
                                    nc.vector.tensor_copy(lA[:], pt[:])
                                else:
                                    nc.vector.tensor_tensor(
                                        lA[:], lA[:], pt[:], op=ADD)
                            cxa = cxpool.tile([P, 512], F16, tag=f"cxA{h}",
                                              bufs=1, name=f"cxA{h}")
                            nc.vector.tensor_copy(cxa[:], cA[:])
                            ctxA[h] = cxa
                            laccA[h] = lA
                        for tt in range(4):
                            va = v_chain(nb + 1, tt, xt_next,
                                         ("b1", "b2")[tt % 2])
                            nc.scalar.copy(v_sb[(nb + 1) * 4 + tt][:], va[:])
                        vaccs = {}
                        partA_cur = (ctxA, laccA)
                xt = xt_next

    nc.compile()
    return nc


def _build_kernel():
    if "k" not in _CACHE:
        _CACHE["k"] = _build()
    return _CACHE["k"]


def prepare_in_maps(x, W_qkv, W_o, cos, sin):
    f16 = np.float16
    tri01 = (np.arange(P)[:, None] <= np.arange(P)[None, :]).astype(f16)
    ones = np.ones((P, P), dtype=f16)
    eye = np.eye(P, dtype=f16)
    cosT = np.ascontiguousarray(cos.T).astype(f16)
    # rotate_half sign folded in: rows (head dims) 0..63 negated
    sgn = np.where(np.arange(P) < P // 2, -1.0, 1.0).astype(np.float32)
    sinT = (sin.T * sgn[:, None]).astype(f16)

    in_maps = []
    for core in range(NCORES):
        b = core // 4
        hg0 = (core % 4) * HPC
        rows = slice(hg0 * P, (hg0 + HPC) * P)
        xT = x[b].T  # [C, T]
        xg = np.ascontiguousarray(
            xT.reshape(NKT, P, T).transpose(1, 0, 2)).astype(f16)

        def wprep(w):  # [512 rows, 2048 c] -> [P, NKT, 512]
            return np.ascontiguousarray(
                w.T.reshape(NKT, P, HPC * P).transpose(1, 0, 2)).astype(f16)

        wq_t = wprep(W_qkv[0 * C:1 * C][rows])
        wk_t = wprep(W_qkv[1 * C:2 * C][rows])
        wv_t = wprep(W_qkv[2 * C:3 * C][rows])
        wo_t = np.ascontiguousarray(
            W_o[:, rows].T.reshape(HPC, P, C).transpose(1, 0, 2)).astype(f16)
        in_maps.append({
            "xg": xg, "wq": wq_t, "wk": wk_t, "wv": wv_t, "wo": wo_t,
            "cosT": cosT, "sinT": sinT, "tri": tri01, "ones": ones,
            "eye": eye,
        })
    return in_maps


def gather(results, b_o):
    y = np.zeros((2, T, C), dtype=np.float32)
    for core in range(NCORES):
        y[core // 4] += results[core]["y"].astype(np.float32)
    y += np.asarray(b_o, dtype=np.float32)[None, None, :]
    return y


def kernel(x, W_qkv, W_o, b_o, cos, sin):
    x = np.asarray(x, dtype=np.float32)
    W_qkv = np.asarray(W_qkv, dtype=np.float32)
    W_o = np.asarray(W_o, dtype=np.float32)
    cos = np.asarray(cos, dtype=np.float32)
    sin = np.asarray(sin, dtype=np.float32)
    nc = _build_kernel()
    in_maps = prepare_in_maps(x, W_qkv, W_o, cos, sin)
    res = run_bass_kernel_spmd(nc, in_maps, core_ids=list(range(NCORES)))
    return gather(res.results, b_o)


# revision 55
# speedup vs baseline: 1.0019x; 1.0019x over previous
"""Causal multi-head attention (RoPE) on 8 TRN2 NeuronCores.

Problem: x[2,2048,2048] -> qkv proj -> rope -> causal attention (16 heads,
head_dim 128) -> output proj + bias. Sharding: (batch, head-group) across the
8 cores - core c handles batch c//4 and heads 4*(c%4)..4*(c%4)+3. Each core
computes a partial output projection over its heads' channels; the host sums
the 4 partials per batch and adds b_o.

Single-pass token-outer pipeline, everything fp16 on device (PSUM accumulation
stays f32; final host reduction in f32; validated rel err ~5e-4 vs the fp32
reference). The exp throughput on ACT (0.833ns/col) exactly matches the
scores+AV cost on PE, so attention phases are ACT-bound unless PE borrows
other work: the next block's QKV projection is software-pipelined INTO the
attention window as three waves:

    attn(nb) heads -> q-wave(nb+1) -> outproj(nb) -> k-wave(nb+1)
                   -> v-wave(nb+1) -> attn(nb+1) ...

Waves are accumulator-major (16 kt matmuls per PSUM bank) with a bank map
chosen so each wave's first banks were freed earliest by the previous phase:
q-wave on b1,b2,b0,b7 / k-wave on b3..b6 / v-wave on b0,b7,b1,b2; attention
rotates scores over b3..b6 4-deep (tag b{3+(i+h)%4}), softmax-denominator
broadcast lb takes the next slot in that rotation, ctx alternates b0/b7.

Scores are transposed s^T[tk,tq] (lhsT=k tile, rhs=q block) with causal
narrowing; matmul cost here is (moving columns) x (cycles/row keyed on the
MOVING operand dtype): fp16 runs 1 cycle/row with no 256-column minimum, so
the r=3 diagonal tile narrows to 128 columns. Softmax denominators come from
element-wise fp16 accumulation of the exp tiles on DVE (2x mode) + ONE
ones-matmul per (head, block) that broadcasts the partition sum - the
per-tile [1,512] ones-matmuls this replaces cost a full 30us of PE. RoPE is
applied in place (half-swap via 2 small SBUF DMAs, sign folded into sinT on
the host; mults split Pool/DVE). Output projection accumulates the 4 heads
in PSUM per 128-token sub-tile; quarter evictions alternate DVE/ACT into an
fp16 [128,2048] staging row, one DMA per sub-tile (per-quarter DMAs on the
last block to shorten the tail). DMAs are batched multi-kt loads; x for block
nb+1 prefetches during attention nb.
"""
import math

import numpy as np

import concourse.bacc as bacc
import concourse.mybir as mybir
import concourse.tile as tile
from concourse.bass_utils import run_bass_kernel_spmd

P = 128           # partitions / head_dim
T = 2048          # context length
C = 2048          # d_model
NKT = C // P      # 16 contraction tiles
NB = T // 512     # 4 token blocks of 512
HPC = 4           # heads per core
NCORES = 8
SCALE = 1.0 / math.sqrt(P)

F32 = mybir.dt.float32
F16 = mybir.dt.float16
EXP = mybir.ActivationFunctionType.Exp
MULT = mybir.AluOpType.mult
ADD = mybir.AluOpType.add

QBANKS = ("b1", "b2", "b0", "b7")   # q-wave accumulators, emission order
KBANKS = ("b3", "b4", "b5", "b6")   # k-wave accumulators
VBANKS = ("b0", "b7", "b1", "b2")   # v-wave accumulators
CTXBANKS = ("b0", "b7")             # ctx_ps alternates by head parity

_CACHE = {}


def _build():
    nc = bacc.Bacc("TRN2", target_bir_lowering=False, debug=False,
                   num_devices=NCORES)
    xg = nc.dram_tensor("xg", (P, NKT, T), F16, kind="ExternalInput").ap()
    wq = nc.dram_tensor("wq", (P, NKT, HPC * P), F16, kind="ExternalInput").ap()
    wk = nc.dram_tensor("wk", (P, NKT, HPC * P), F16, kind="ExternalInput").ap()
    wv = nc.dram_tensor("wv", (P, NKT, HPC * P), F16, kind="ExternalInput").ap()
    wo = nc.dram_tensor("wo", (P, HPC, C), F16, kind="ExternalInput").ap()
    cosT = nc.dram_tensor("cosT", (P, T), F16, kind="ExternalInput").ap()
    sinT = nc.dram_tensor("sinT", (P, T), F16, kind="ExternalInput").ap()
    tri = nc.dram_tensor("tri", (P, P), F16, kind="ExternalInput").ap()
    ones = nc.dram_tensor("ones", (P, P), F16, kind="ExternalInput").ap()
    eye = nc.dram_tensor("eye", (P, P), F16, kind="ExternalInput").ap()
    y = nc.dram_tensor("y", (T, C), F16, kind="ExternalOutput").ap()

    half = P // 2

    with tile.TileContext(nc) as tc:
        with (
            tc.tile_pool(name="gconst", bufs=1) as gpool,
            tc.tile_pool(name="wbuf", bufs=1) as wpool,
            tc.tile_pool(name="xbuf", bufs=1) as xpool,
            tc.tile_pool(name="qkbuf", bufs=1) as qkpool,
            tc.tile_pool(name="vbuf", bufs=1) as vpool,
            tc.tile_pool(name="rope", bufs=1) as rpool,
            tc.tile_pool(name="ptb", bufs=1) as ptpool,
            tc.tile_pool(name="stats", bufs=1) as spool,
            tc.tile_pool(name="ctxb", bufs=1) as cxpool,
            tc.tile_pool(name="yb", bufs=1) as ypool,
            tc.tile_pool(name="ps", bufs=1, space="PSUM") as ps,
        ):
            tri_sb = gpool.tile([P, P], F16, tag="tri")
            ones_sb = gpool.tile([P, P], F16, tag="ones")
            eye_sb = gpool.tile([P, P], F16, tag="eye")
            wq_sb = wpool.tile([P, NKT, HPC * P], F16, tag="wq", name="wq_sb")
            wk_sb = wpool.tile([P, NKT, HPC * P], F16, tag="wk", name="wk_sb")
            wv_sb = wpool.tile([P, NKT, HPC * P], F16, tag="wv", name="wv_sb")
            wo_sb = wpool.tile([P, HPC, C], F16, tag="wo", name="wo_sb")
            cos_sb = wpool.tile([P, T], F16, tag="cos", name="cos_sb")
            sin_sb = wpool.tile([P, T], F16, tag="sin", name="sin_sb")

            qk_sb = {}
            for h in range(HPC):
                for part in ("q", "k"):
                    for nb in range(NB):
                        qk_sb[(part, h, nb)] = qkpool.tile(
                            [P, 512], F16, tag=f"{part}{h}n{nb}",
                            name=f"{part}{h}n{nb}_sb")
            v_sb = [vpool.tile([P, 512], F16, tag=f"vb{i}", name=f"v{i}_sb")
                    for i in range(NKT)]



            def wave_accs(banks, label):
                return [ps.tile([P, 512], F32, tag=banks[h],
                                name=f"{label}{h}") for h in range(HPC)]

            def rope_one(nb, part, h):
                """In-place rope on an evicted q/k chunk. Emitted away from
                the attention masks: the Pool t1 multiply is 1.1us, and a
                diagonal mask queued behind it stalls the AV matmuls."""
                nsl = slice(nb * 512, (nb + 1) * 512)
                dst = qk_sb[(part, h, nb)]
                tmp = rpool.tile([P, 512], F16, tag="rt", bufs=2, name="rtmp")
                nc.sync.dma_start(tmp[0:half, :], dst[half:P, :])
                nc.sync.dma_start(tmp[half:P, :], dst[0:half, :])
                t1 = rpool.tile([P, 512], F16, tag="t1", bufs=2)
                nc.gpsimd.tensor_tensor(t1[:], dst[:], cos_sb[:, nsl], op=MULT)
                t2 = rpool.tile([P, 512], F16, tag="t2", bufs=2)
                nc.vector.tensor_tensor(t2[:], tmp[:], sin_sb[:, nsl], op=MULT)
                nc.vector.tensor_tensor(dst[:], t1[:], t2[:], op=ADD)

            def evict_rope_one(nb, part, h, acc_ap):
                nc.scalar.copy(qk_sb[(part, h, nb)][:], acc_ap)
                rope_one(nb, part, h)

            def qk_wave_chain(part, xt, h, bank):
                wsb = wq_sb if part == "q" else wk_sb
                acc = ps.tile([P, 512], F32, tag=bank, name=f"{part}{h}")
                for kt in range(NKT):
                    nc.tensor.matmul(
                        acc[:], wsb[:, kt, h * P:(h + 1) * P],
                        xt[:, kt, :], start=(kt == 0), stop=(kt == NKT - 1))
                return acc

            def v_chain(nb, tt, xt, bank):
                vacc = ps.tile([P, 512], F32, tag=bank, name=f"v{nb}_{tt}")
                for kt in range(NKT):
                    nc.tensor.matmul(
                        vacc[:], xt[:, kt, tt * P:(tt + 1) * P],
                        wv_sb[:, kt, :], start=(kt == 0), stop=(kt == NKT - 1))
                return vacc

            def attention(nb, xt_next, vaccs, partA=None):
                """vaccs: this block's un-evicted v accumulators (tt 1..3 on
                b1,b2,b7); evictions are emitted just-in-time at the diagonal
                steps of head 0 so ACT serves head 0's first exps first. For
                nb==3 the tt>=1 v chains are emitted inside head 0 as PE
                filler (no next-block waves exist). Head h's softmax stats
                (lb matmul, reciprocal, normalize) are deferred into head
                h+1's pipeline so PE never waits on the DVE denominator
                chain. For nb<3 the next block's q-wave chain for head h is
                emitted right after head h (banks b1/b2 alternating, evicted
                and roped immediately)."""
                nt = 4 * (nb + 1)
                ctx_tiles = {}
                pending = None

                def q_filler():
                    # next block's q projection, one matmul per drain unit,
                    # each chain evicted (ACT) as soon as it completes
                    for fh in range(HPC):
                        acc = ps.tile([P, 512], F32, tag=f"b{1 + fh % 2}",
                                      name=f"q{fh}")
                        for kt in range(NKT):
                            nc.tensor.matmul(
                                acc[:], wq_sb[:, kt, fh * P:(fh + 1) * P],
                                xt_next[:, kt, :], start=(kt == 0),
                                stop=(kt == NKT - 1))
                            yield
                        nc.scalar.copy(qk_sb[("q", fh, nb + 1)][:], acc[:])

                filler = q_filler() if nb < NB - 1 else None

                def drain(n):
                    if filler is None:
                        return
                    for _ in range(n):
                        if next(filler, "done") == "done":
                            break

                per_step = max(1, 58 // (3 * nt))

                def stats(h, ctx_ps, lacc):
                    # slot (h+3)%4 is the one head h+1 touches last after the
                    # deferred emission point, so the reciprocal drains
                    # before the bank is needed again
                    lbt = ps.tile([P, 512], F32, tag=f"b{3 + (h + 3) % 4}",
                                  name=f"l{h}_{nb}")
                    nc.tensor.matmul(lbt[:], ones_sb[:], lacc[:],
                                     start=True, stop=True)
                    rinv = spool.tile([P, 512], F32, tag="rinv", bufs=2)
                    ctx_sb = cxpool.tile([P, 512], F16, tag=f"cx{h}", bufs=2,
                                         name=f"cs{h}_{nb}")
                    # last head's normalize gates the output projection: do it
                    # in chunks so outproj's first sub-tile unblocks early
                    for lo, hi in ((0, 128), (128, 256), (256, 512)) \
                            if h == HPC - 1 else ((0, 512),):
                        nc.vector.reciprocal(rinv[:, lo:hi], lbt[:, lo:hi])
                        nc.vector.tensor_tensor(ctx_sb[:, lo:hi],
                                                ctx_ps[:, lo:hi],
                                                rinv[:, lo:hi], op=MULT)
                    ctx_tiles[h] = ctx_sb

                i0 = 8 if partA is not None else 0
                pend_at = i0 + (4 if nt - i0 > 4 else 3)
                for h in range(HPC):
                    qT = qk_sb[("q", h, nb)]
                    ctx_ps = ps.tile([P, 512], F32, tag=CTXBANKS[h % 2],
                                     name=f"ctx{h}_{nb}")
                    lacc = spool.tile([P, 512], F16, tag="lacc", bufs=2)
                    if partA is not None:
                        # resume this head's accumulation from the partial
                        # computed in the wave window: identity-matmul the
                        # parked ctx back into PSUM (opens the group), start
                        # the denominator from the parked lacc
                        ctxA, laccA = partA
                        nc.tensor.matmul(ctx_ps[:], eye_sb[:], ctxA[h][:],
                                         start=True, stop=False)
                        nc.vector.tensor_copy(lacc[:], laccA[h][:])
                    for i in range(i0, nt):
                        r = i - 4 * nb
                        if h == 0 and r >= 1 and r in vaccs:
                            nc.scalar.copy(v_sb[nb * 4 + r][:], vaccs[r][:])
                        if h > 0 and i == pend_at and pending is not None:
                            stats(*pending)
                            pending = None
                        c0 = 0 if r < 1 else r * P
                        osl = slice(c0, 512)
                        sps = ps.tile([P, 512], F32, tag=f"b{3 + (i + h) % 4}",
                                      name=f"s{h}_{nb}_{i}")
                        nc.tensor.matmul(
                            sps[:, osl],
                            qk_sb[("k", h, i // 4)][:,
                                                    (i % 4) * P:(i % 4 + 1) * P],
                            qT[:, osl], start=True, stop=True)
                        pt = ptpool.tile([P, 512], F16, tag="pt", bufs=6)
                        nc.scalar.activation(pt[:, osl], sps[:, osl], EXP,
                                             scale=SCALE)
                        if r >= 0:
                            # diagonal mask on DVE: fp16 2x mode takes 127ns
                            # vs Pool's 444ns+launch, and it's on the exp->AV
                            # critical path every diagonal step
                            dsl = slice(r * P, (r + 1) * P)
                            nc.vector.tensor_tensor(
                                pt[:, dsl], pt[:, dsl], tri_sb[:], op=MULT)
                        nc.tensor.matmul(
                            ctx_ps[:, osl],
                            v_sb[i][:, h * P:(h + 1) * P], pt[:, osl],
                            start=(i == 0 and partA is None),
                            stop=(i == nt - 1))
                        if i == i0 and partA is None:
                            nc.vector.tensor_copy(lacc[:], pt[:])
                        else:
                            nc.vector.tensor_tensor(
                                lacc[:, osl], lacc[:, osl], pt[:, osl], op=ADD)
                        if h >= 1:
                            drain(per_step)
                    if h == HPC - 1:
                        if pending is not None:
                            stats(*pending)
                            pending = None
                        drain(6)
                        stats(h, ctx_ps, lacc)
                        drain(NKT * HPC)
                    else:
                        pending = (h, ctx_ps, lacc)
                if nb < NB - 1:
                    for h in range(HPC):
                        rope_one(nb + 1, "q", h)
                return ctx_tiles

            def outproj(nb, ctx_tiles):
                last = nb == NB - 1

                def make_yps(ob, name):
                    # 4-bank ring absorbs eviction latency; b1 frees earliest
                    # for the k-wave that follows
                    t = ps.tile([P, 512], F32, tag=("b1", "b2", "b3", "b4")[ob],
                                name=name)
                    return lambda lo=0, hi=512: t[:, lo:hi]

                for sub in range(4):
                    tt = nb * 4 + sub
                    ssl = slice(sub * P, (sub + 1) * P)
                    ysb = ypool.tile([P, C], F16, tag="ysb", bufs=3)
                    ypsf = {}
                    if last and sub == 0:
                        # on the final block the h3 normalize is still in
                        # flight; open all 4 accumulation groups with their
                        # h0..h2 contributions first to cover its latency
                        for ob in range(NB):
                            obsl = slice(ob * 512, (ob + 1) * 512)
                            ypsf[ob] = make_yps(ob, f"y{tt}_{ob}")
                            for h in range(HPC - 1):
                                nc.tensor.matmul(
                                    ypsf[ob](), ctx_tiles[h][:, ssl],
                                    wo_sb[:, h, obsl],
                                    start=(h == 0), stop=False,
                                    skip_group_check=True)
                    for ob in range(NB):
                        obsl = slice(ob * 512, (ob + 1) * 512)
                        if ob in ypsf:
                            f = ypsf[ob]
                            nc.tensor.matmul(
                                f(), ctx_tiles[HPC - 1][:, ssl],
                                wo_sb[:, HPC - 1, obsl],
                                start=False, stop=True,
                                skip_group_check=True)
                        else:
                            f = make_yps(ob, f"y{tt}_{ob}")
                            halves = ((0, 512),)
                            if last and sub == 3 and ob == 3:
                                # split the final group so its first half's
                                # eviction/DMA overlaps the second half
                                halves = ((0, 256), (256, 512))
                            for lo, hi in halves:
                                for h in range(HPC):
                                    nc.tensor.matmul(
                                        f(lo, hi), ctx_tiles[h][:, ssl],
                                        wo_sb[:, h, ob * 512 + lo:
                                              ob * 512 + hi],
                                        start=(h == 0), stop=(h == HPC - 1),
                                        skip_group_check=True)
                                if hi - lo < 512:
                                    nc.vector.tensor_copy(
                                        ysb[:, ob * 512 + lo:ob * 512 + hi],
                                        f(lo, hi))
                                    nc.sync.dma_start(
                                        y[tt * P:(tt + 1) * P,
                                          ob * 512 + lo:ob * 512 + hi],
                                        ysb[:, ob * 512 + lo:ob * 512 + hi])
                        if last and sub == 3 and ob == 3:
                            continue
                        if ob % 2 == 0:
                            nc.vector.tensor_copy(ysb[:, obsl], f())
                        else:
                            nc.scalar.copy(ysb[:, obsl], f())
                        if last:
                            nc.sync.dma_start(
                                y[tt * P:(tt + 1) * P, obsl], ysb[:, obsl])
                    if not last:
                        nc.sync.dma_start(y[tt * P:(tt + 1) * P, :], ysb[:])

            # ---- startup DMAs: x(0)+wq+wk interleaved, fine-grained head ----
            x0 = xpool.tile([P, NKT, 512], F16, tag="x", bufs=2, name="x_nb0")
            groups = [(0, 1), (1, 2), (2, 4), (4, 8), (8, 12), (12, 16)]
            for lo, hi in groups:
                gs = slice(lo, hi)
                nc.sync.dma_start(x0[:, gs, :], xg[:, gs, 0:512])
                nc.sync.dma_start(wq_sb[:, gs, :], wq[:, gs, :])
                nc.sync.dma_start(wk_sb[:, gs, :], wk[:, gs, :])
            nc.sync.dma_start(cos_sb[:], cosT)
            nc.sync.dma_start(sin_sb[:], sinT)
            nc.sync.dma_start(wv_sb[:], wv)
            nc.sync.dma_start(tri_sb[:], tri)
            nc.sync.dma_start(ones_sb[:], ones)
            nc.sync.dma_start(eye_sb[:], eye)
            nc.sync.dma_start(wo_sb[:], wo)

            def v_group(nbv, xt_src):
                """tt0 evicted immediately; tt1..3 stay in PSUM for the JIT
                evictions inside attention(nbv) head 0."""
                va0 = v_chain(nbv, 0, xt_src, "b1")
                nc.scalar.copy(v_sb[nbv * 4][:], va0[:])
                return {tt: v_chain(nbv, tt, xt_src, ("b1", "b2", "b7")[tt - 1])
                        for tt in (1, 2, 3)}

            # ---- block 0: plain qk (kt-outer tracks DMA arrival), v ----
            qaccs = wave_accs(QBANKS, "q")
            kaccs = wave_accs(KBANKS, "k")
            for kt in range(NKT):
                for part, wsb, accs in (("q", wq_sb, qaccs),
                                        ("k", wk_sb, kaccs)):
                    for h in range(HPC):
                        nc.tensor.matmul(
                            accs[h][:], wsb[:, kt, h * P:(h + 1) * P],
                            x0[:, kt, :], start=(kt == 0), stop=(kt == NKT - 1))
            for h in range(HPC):
                evict_rope_one(0, "k", h, kaccs[h][:])
            for h in range(HPC):
                evict_rope_one(0, "q", h, qaccs[h][:])
            vaccs = v_group(0, x0)

            xt = x0
            partA_cur = None
            for nb in range(NB):
                # prefetch next block's x during attention
                if nb < NB - 1:
                    xt_next = xpool.tile([P, NKT, 512], F16, tag="x", bufs=2,
                                         name=f"x_nb{nb + 1}")
                    nsl2 = slice((nb + 1) * 512, (nb + 2) * 512)
                    # halves: the first q-filler chains only need low kt
                    nc.sync.dma_start(xt_next[:, 0:8, :], xg[:, 0:8, nsl2])
                    nc.sync.dma_start(xt_next[:, 8:16, :], xg[:, 8:16, nsl2])
                else:
                    xt_next = xt
                ctx_tiles = attention(nb, xt_next, vaccs, partA=partA_cur)
                outproj(nb, ctx_tiles)
                if nb < NB - 1:
                    if nb + 1 < NB - 1:
                        for h in range(HPC):
                            ka = qk_wave_chain("k", xt_next, h,
                                               f"b{1 + h % 2}")
                            evict_rope_one(nb + 1, "k", h, ka[:])
                        vaccs = v_group(nb + 1, xt_next)
                    else:
                        # the last block's attention is ACT(exp)-bound, so
                        # compute its first two key-blocks per head NOW,
                        # interleaved with the k-wave, where ACT has slack;
                        # partial ctx/lacc park in SBUF (fp16) and resume in
                        # attention(3) via an identity-matmul preload
                        ctxA, laccA = {}, {}
                        for h in range(HPC):
                            ka = qk_wave_chain("k", xt_next, h,
                                               f"b{1 + h % 2}")
                            evict_rope_one(nb + 1, "k", h, ka[:])
                            qT = qk_sb[("q", h, nb + 1)]
                            cA = ps.tile([P, 512], F32, tag=CTXBANKS[h % 2],
                                         name=f"cA{h}")
                            lA = spool.tile([P, 512], F16, tag=f"lA{h}",
                                            bufs=1)
                            for i in range(8):
                                sA = ps.tile([P, 512], F32,
                                             tag=f"b{3 + (i + h) % 4}",
                                             name=f"sA{h}_{i}")
                                nc.tensor.matmul(
                                    sA[:],
                                    qk_sb[("k", h, i // 4)][:,
                                                            (i % 4) * P:
                                                            (i % 4 + 1) * P],
                                    qT[:], start=True, stop=True)
                                pt = ptpool.tile([P, 512], F16, tag="pt",
                                                 bufs=6)
                                nc.scalar.activation(pt[:], sA[:], EXP,
                                                     scale=SCALE)
                                nc.tensor.matmul(
                                    cA[:], v_sb[i][:, h * P:(h + 1) * P],
                                    pt[:], start=(i == 0), stop=(i == 7))
                                if i == 0:
                                    nc.vector.tensor_copy(lA[:], pt[:])
                                else:
                                    nc.vector.tensor_tensor(
                                        lA[:], lA[:], pt[:], op=ADD)
                            cxa = cxpool.tile([P, 512], F16, tag=f"cxA{h}",
                                              bufs=1, name=f"cxA{h}")
                            nc.vector.tensor_copy(cxa[:], cA[:])
                            ctxA[h] = cxa
                            laccA[h] = lA
                        for tt in range(4):
                            va = v_chain(nb + 1, tt, xt_next,
                                         ("b1", "b2")[tt % 2])
                            nc.scalar.copy(v_sb[(nb + 1) * 4 + tt][:], va[:])
                        vaccs = {}
                        partA_cur = (ctxA, laccA)
                xt = xt_next

    nc.compile()
    return nc


def _build_kernel():
    if "k" not in _CACHE:
        _CACHE["k"] = _build()
    return _CACHE["k"]


def prepare_in_maps(x, W_qkv, W_o, cos, sin):
    f16 = np.float16
    tri01 = (np.arange(P)[:, None] <= np.arange(P)[None, :]).astype(f16)
    ones = np.ones((P, P), dtype=f16)
    eye = np.eye(P, dtype=f16)
    cosT = np.ascontiguousarray(cos.T).astype(f16)
    # rotate_half sign folded in: rows (head dims) 0..63 negated
    sgn = np.where(np.arange(P) < P // 2, -1.0, 1.0).astype(np.float32)
    sinT = (sin.T * sgn[:, None]).astype(f16)

    in_maps = []
    for core in range(NCORES):
        b = core // 4
        hg0 = (core % 4) * HPC
        rows = slice(hg0 * P, (hg0 + HPC) * P)
        xT = x[b].T  # [C, T]
        xg = np.ascontiguousarray(
            xT.reshape(NKT, P, T).transpose(1, 0, 2)).astype(f16)

        def wprep(w):  # [512 rows, 2048 c] -> [P, NKT, 512]
            return np.ascontiguousarray(
                w.T.reshape(NKT, P, HPC * P).transpose(1, 0, 2)).astype(f16)

        wq_t = wprep(W_qkv[0 * C:1 * C][rows])
        wk_t = wprep(W_qkv[1 * C:2 * C][rows])
        wv_t = wprep(W_qkv[2 * C:3 * C][rows])
        wo_t = np.ascontiguousarray(
            W_o[:, rows].T.reshape(HPC, P, C).transpose(1, 0, 2)).astype(f16)
        in_maps.append({
            "xg": xg, "wq": wq_t, "wk": wk_t, "wv": wv_t, "wo": wo_t,
            "cosT": cosT, "sinT": sinT, "tri": tri01, "ones": ones,
            "eye": eye,
        })
    return in_maps


def gather(results, b_o):
    y = np.zeros((2, T, C), dtype=np.float32)
    for core in range(NCORES):
        y[core // 4] += results[core]["y"].astype(np.float32)
    y += np.asarray(b_o, dtype=np.float32)[None, None, :]
    return y


def kernel(x, W_qkv, W_o, b_o, cos, sin):
    x = np.asarray(x, dtype=np.float32)
    W_qkv = np.asarray(W_qkv, dtype=np.float32)
    W_o = np.asarray(W_o, dtype=np.float32)
    cos = np.asarray(cos, dtype=np.float32)
    sin = np.asarray(sin, dtype=np.float32)
    nc = _build_kernel()
    in_maps = prepare_in_maps(x, W_qkv, W_o, cos, sin)
    res = run_bass_kernel_spmd(nc, in_maps, core_ids=list(range(NCORES)))
    return gather(res.results, b_o)
